# revision 19
# baseline (speedup 1.0000x reference)
"""Trainium2 Bass kernel: Mistral quantized MLP (SwiGLU with int8-valued int32
weights, per-output-channel scales).

  gate = (x @ dequant(gate_wq).T), up = (x @ dequant(up_wq).T)
  h = silu(gate) * up
  out = h @ dequant(down_wq).T

Strategy (8 NeuronCores, tensor-parallel on the intermediate dim I):
  - Core c owns rows [c*I/8, (c+1)*I/8) of gate/up and the matching columns of
    down. Each core computes a full [H, T] partial of the down projection in
    f16; the host sums the 8 partials in f32 (the "all-reduce"), applies
    down_scale, and transposes back to [B, S, H].

  - Precision: fp16 activations with exactly-represented int8 weights for most
    of the contraction; a calibrated subset of k-tiles runs as fp8(e4m3)
    DoubleRow matmuls (2 k-tiles per ~518-cycle instruction = 2x PE rate).
    The 2e-2 rel-l2 budget is spent to saturation: gate gets 4 fp8 tiles of
    32, up gets 6 on the first 7 of 14 out-groups and 4 on the rest
    (sub-tile-granular allocation; sim-calibrated to 1.917e-2).

  - Layout: features on partitions, tokens on free dim. x pre-tiled on host to
    [mega, ki, ko, t] f16 plus an fp8 tail [mega, ki, n8x, t] (value 8*x);
    weights to [o_tile, ki, ko, o] (fp8 value w/8). Each matmul is
    lhsT=[128k, 128o] stationary x rhs=[128k, 512t] moving, fp32 PSUM.

  - Startup: mega 0's x arrives as 7 chunk DMAs on the Sync queue while the
    first gate/up weight tiles arrive as 4-k-tile chunks issued from the
    Scalar/Vector queues, so the first matmul starts ~10us after the NEFF
    preamble instead of ~23us (DMA issue slots are ~0.6us each per queue and
    per-queue bandwidth is ~80GB/s; splitting engines parallelizes both).
"""

import os

import ml_dtypes
import numpy as np

_E4 = ml_dtypes.float8_e4m3

# Problem dims (hardcoded per the task contract).
B, S, H, I = 2, 2048, 4096, 14336
NCORES = 8
I_LOC = I // NCORES  # 1792
T = B * S  # 4096
T_MEGA = 512  # tokens per resident x block (and per-matmul free dim)

# fp8 k-tile allocation: gate (of 32) and up (of 32), both even.
# The first GA up-groups (of 14 per core) get N8UA tiles, the rest N8U.
N8G = int(os.environ.get("TRNMLP_N8G", "2"))
N8U = int(os.environ.get("TRNMLP_N8U", "6"))
N8UA = int(os.environ.get("TRNMLP_N8UA", "8"))
GA = int(os.environ.get("TRNMLP_GA", "12"))

_nc_cache = {}


def _build_module(t_mega, n_mega, ko_g, ot_g, ot_d, n8g, n8u, n8ua, ga):
    """Build + compile the (SPMD, identical on all cores) Bass module.

    ko_g: contraction tiles for gate/up (H/128 = 32)
    ot_g: output tiles per core for gate/up (I_loc/128 = 14); also the down
          contraction tile count
    ot_d: output tiles for down (H/128 = 32)
    n8g: gate fp8 tail tiles; n8ua/n8u: up fp8 tail tiles for groups
         [0, ga) / [ga, ot_g)
    """
    import concourse.tile as tile
    from concourse import bacc, mybir

    f32 = mybir.dt.float32
    f16 = mybir.dt.float16
    f8 = mybir.dt.float8e4
    silu = mybir.ActivationFunctionType.Silu
    mult = mybir.AluOpType.mult
    DR = mybir.MatmulPerfMode.DoubleRow
    assert n8g % 2 == 0 and n8u % 2 == 0 and n8ua % 2 == 0
    assert n8ua >= n8u and 0 <= ga <= ot_g
    n8x = max(n8g, n8ua if ga else n8u)
    ko_x = ko_g - min(n8g, n8u)  # fp16 x tiles needed
    kg16 = ko_g - n8g  # fp16 k-tiles for gate

    nc = bacc.Bacc(
        "TRN2",
        target_bir_lowering=False,
        debug=False,
        enable_asserts=False,
        num_devices=NCORES,
    )

    xh_d = nc.dram_tensor(
        "x_hi", [n_mega, 128, ko_x, t_mega], f16, kind="ExternalInput"
    ).ap()
    x8_d = nc.dram_tensor(
        "x8", [n_mega, 128, n8x, t_mega], f8, kind="ExternalInput"
    ).ap()
    gw_d = nc.dram_tensor(
        "gate_w", [ot_g, 128, kg16, 128], f16, kind="ExternalInput"
    ).ap()
    gw8_d = nc.dram_tensor(
        "gate_w8", [ot_g, 128, n8g, 128], f8, kind="ExternalInput"
    ).ap()
    uwA_d = uw8A_d = None
    if ga:
        uwA_d = nc.dram_tensor(
            "up_wA", [ga, 128, ko_g - n8ua, 128], f16, kind="ExternalInput"
        ).ap()
        uw8A_d = nc.dram_tensor(
            "up_w8A", [ga, 128, n8ua, 128], f8, kind="ExternalInput"
        ).ap()
    uwB_d = uw8B_d = None
    if ga < ot_g:
        uwB_d = nc.dram_tensor(
            "up_wB", [ot_g - ga, 128, ko_g - n8u, 128], f16,
            kind="ExternalInput",
        ).ap()
        uw8B_d = nc.dram_tensor(
            "up_w8B", [ot_g - ga, 128, n8u, 128], f8, kind="ExternalInput"
        ).ap()
    dw_d = nc.dram_tensor(
        "down_w", [ot_d, 128, ot_g, 128], f16, kind="ExternalInput"
    ).ap()
    gs_d = nc.dram_tensor("gate_s", [128, ot_g], f32, kind="ExternalInput").ap()
    us_d = nc.dram_tensor("up_s", [128, ot_g], f32, kind="ExternalInput").ap()
    out_d = nc.dram_tensor(
        "out", [ot_d * 128, n_mega * t_mega], f16, kind="ExternalOutput"
    ).ap()

    # mega-0 x chunk sizes (prefix consumed first; small leading chunks so
    # the first matmul starts early) and first-group weight chunk sizes.
    rem = ko_x - 16
    XCH = [2, 2, 4, 4, 4] + ([rem - rem // 2, rem // 2] if rem > 0 else [])
    assert sum(XCH) == ko_x
    GCH = [4] * (kg16 // 4) + ([kg16 % 4] if kg16 % 4 else [])
    assert sum(GCH) == kg16
    ku16_0 = ko_g - (n8ua if ga else n8u)
    UCH = [4] * (ku16_0 // 4) + ([ku16_0 % 4] if ku16_0 % 4 else [])

    with tile.TileContext(nc) as tc:
        with (
            tc.tile_pool(name="px", bufs=2) as px,
            tc.tile_pool(name="px0", bufs=1) as px0,
            tc.tile_pool(name="pw", bufs=2) as pw,
            tc.tile_pool(name="pw0", bufs=1) as pw0,
            tc.tile_pool(name="pdw", bufs=4) as pdw,
            tc.tile_pool(name="ph", bufs=2) as ph,
            tc.tile_pool(name="pe", bufs=2) as pe,
            tc.tile_pool(name="po", bufs=2) as po,
            tc.tile_pool(name="pscale", bufs=1) as pscale,
            tc.tile_pool(name="pp", bufs=8, space="PSUM") as pp,
        ):
            # scales issued from the GpSimd queue (needed only at the first
            # activation, ~20us in; keeps the Sync queue for x chunks)
            gs_t = pscale.tile([128, ot_g], f32, name="gs_t")
            nc.gpsimd.dma_start(out=gs_t[:], in_=gs_d[:])
            us_t = pscale.tile([128, ot_g], f32, name="us_t")
            nc.gpsimd.dma_start(out=us_t[:], in_=us_d[:])

            # HAM pre-warm: the PE clock-gate opens only after ~3.4us of
            # sustained activity. Run a chain of dummy FD=64 matmuls on
            # zeroed tiles while the first x/weight DMAs are in flight so the
            # real matmul stream starts at 2.4GHz instead of 1.2GHz.
            dmw = pscale.tile([128, 128], f16, name="dmw")
            nc.vector.memset(dmw[:], 0.0)
            dmx = pscale.tile([128, 64], f16, name="dmx")
            nc.vector.memset(dmx[:], 0.0)
            pwarm = pp.tile([128, 64], f32, tag="ps", name="pwarm")
            for i in range(64):
                nc.tensor.matmul(
                    pwarm[:], dmw[:], dmx[:], start=(i == 0), stop=(i == 63)
                )

            def chunked(pool, dram_ap, sizes, nk, width, dt, tag, eng):
                """Load dram_ap ([128, nk, width]) as len(sizes) chunk tiles
                on engine eng (or eng[i] per chunk); returns accessor
                k -> AP [128, width]."""
                engs = eng if isinstance(eng, list) else [eng] * len(sizes)
                chunks, s0 = [], 0
                for i, csz in enumerate(sizes):
                    t_ = pool.tile(
                        [128, csz, width], dt, tag=f"{tag}{i}", name=f"{tag}{i}"
                    )
                    engs[i].dma_start(
                        out=t_[:], in_=dram_ap[:, s0 : s0 + csz, :]
                    )
                    chunks.append((t_, s0, csz))
                    s0 += csz
                assert s0 == nk

                def acc(k):
                    for t_, st, csz in chunks:
                        if st <= k < st + csz:
                            return t_[:, k - st, :]
                    raise IndexError(k)

                return acc

            def g_group(m, ot, xh, x8, hh, gwa=None, uwa=None):
                """Gate+up matmul group for (mega m, out tile ot) + SwiGLU."""
                n8u_ot = n8ua if ot < ga else n8u
                ku16 = ko_g - n8u_ot
                if gwa is None:
                    gw = pw.tile([128, kg16, 128], f16, tag="gw", name="gw")
                    nc.sync.dma_start(out=gw[:], in_=gw_d[ot])
                    gwa = lambda k, gw=gw: gw[:, k, :]
                if uwa is None:
                    tag = "uwA" if ot < ga else "uwB"
                    src = uwA_d[ot] if ot < ga else uwB_d[ot - ga]
                    uw = pw.tile([128, ku16, 128], f16, tag=tag, name=tag)
                    nc.sync.dma_start(out=uw[:], in_=src)
                    uwa = lambda k, uw=uw: uw[:, k, :]
                gw8 = pw.tile([128, n8g, 128], f8, tag="gw8", name="gw8")
                nc.sync.dma_start(out=gw8[:], in_=gw8_d[ot])
                tag = "uw8A" if ot < ga else "uw8B"
                src = uw8A_d[ot] if ot < ga else uw8B_d[ot - ga]
                uw8a = pw.tile([128, n8u_ot, 128], f8, tag=tag, name=tag)
                nc.sync.dma_start(out=uw8a[:], in_=src)

                psg = pp.tile([128, t_mega], f32, tag="ps", name="psg")
                for k in range(kg16):
                    nc.tensor.matmul(
                        psg[:], gwa(k), xh(k),
                        start=(k == 0), stop=(n8g == 0 and k == kg16 - 1),
                    )
                for p in range(n8g // 2):
                    o8 = n8x - n8g + 2 * p
                    nc.tensor.matmul(
                        psg[:], gw8[:, 2 * p : 2 * p + 2, :],
                        x8[:, o8 : o8 + 2, :],
                        start=False, stop=(p == n8g // 2 - 1),
                        perf_mode=DR,
                    )
                psu = pp.tile([128, t_mega], f32, tag="ps", name="psu")
                for k in range(ku16):
                    nc.tensor.matmul(
                        psu[:], uwa(k), xh(k),
                        start=(k == 0), stop=(n8u_ot == 0 and k == ku16 - 1),
                    )
                for p in range(n8u_ot // 2):
                    o8 = n8x - n8u_ot + 2 * p
                    nc.tensor.matmul(
                        psu[:], uw8a[:, 2 * p : 2 * p + 2, :],
                        x8[:, o8 : o8 + 2, :],
                        start=False, stop=(p == n8u_ot // 2 - 1),
                        perf_mode=DR,
                    )

                gact = pe.tile([128, t_mega], f32, tag="gact", name="gact")
                nc.scalar.activation(
                    gact[:], psg[:], silu, scale=gs_t[:, ot : ot + 1]
                )
                # h = (up_psum * up_scale) * silu(gate * gate_scale)
                nc.vector.scalar_tensor_tensor(
                    hh[:, ot, :], psu[:], us_t[:, ot : ot + 1], gact[:],
                    mult, mult,
                )

            def d_group(m, o2, hh, split=False):
                """Down matmul group for (mega m, out tile o2); host scales.

                split: evacuate in two half-token chunks so the copy of half 1
                overlaps the DMA of half 0 (used for the final store, which
                sits on the critical path after the last matmul)."""
                dw = pdw.tile([128, ot_g, 128], f16, tag="dw", name="dw")
                nc.sync.dma_start(out=dw[:], in_=dw_d[o2])
                pso = pp.tile([128, t_mega], f32, tag="ps", name="pso")
                for k in range(ot_g):
                    nc.tensor.matmul(
                        pso[:], dw[:, k, :], hh[:, k, :],
                        start=(k == 0), stop=(k == ot_g - 1),
                    )
                nh = 2 if split else 1
                tw = t_mega // nh
                for h_ in range(nh):
                    ob = po.tile(
                        [128, tw], f16, tag=f"ob{nh}{h_}", name=f"ob{nh}{h_}"
                    )
                    nc.scalar.copy(ob[:], pso[:, h_ * tw : (h_ + 1) * tw])
                    nc.sync.dma_start(
                        out=out_d[
                            o2 * 128 : (o2 + 1) * 128,
                            m * t_mega + h_ * tw : m * t_mega + (h_ + 1) * tw,
                        ],
                        in_=ob[:],
                    )

            # Software pipeline: interleave mega m's gate/up groups with mega
            # m-1's down groups, spreading the down-phase DMA (down weights +
            # out stores) across the whole mega so HBM never saturates and the
            # PE never stalls.
            prev = None  # (m-1, hh)
            for m in range(n_mega):
                if m == 0:
                    xh = chunked(
                        px0, xh_d[0], XCH, ko_x, t_mega, f16, "xc", nc.sync
                    )
                    x8 = px.tile([128, n8x, t_mega], f8, tag="x8", name="x8")
                    nc.sync.dma_start(out=x8[:], in_=x8_d[0])
                else:
                    xh_t = px.tile(
                        [128, ko_x, t_mega], f16, tag="xh", name="xh"
                    )
                    nc.sync.dma_start(out=xh_t[:], in_=xh_d[m])

                    def xh(k, xh_t=xh_t):
                        return xh_t[:, k, :]

                    x8 = px.tile([128, n8x, t_mega], f8, tag="x8", name="x8")
                    nc.sync.dma_start(out=x8[:], in_=x8_d[m])
                hh = ph.tile([128, ot_g, t_mega], f16, tag="hh", name="hh")

                for ot in range(ot_g):
                    gwa = uwa = None
                    if m == 0 and ot == 0:
                        # first group: weight chunks from Scalar/Vector queues
                        gwa = chunked(
                            pw0, gw_d[0], GCH, kg16, 128, f16, "gwc",
                            nc.scalar,
                        )
                        uwa = chunked(
                            pw0, (uwA_d if ga else uwB_d)[0], UCH, ku16_0,
                            128, f16, "uwc", nc.gpsimd,
                        )
                    g_group(m, ot, xh, x8, hh, gwa, uwa)
                    if prev is not None:
                        pm, phh = prev
                        for o2 in range(
                            ot_d * ot // ot_g, ot_d * (ot + 1) // ot_g
                        ):
                            d_group(pm, o2, phh)
                prev = (m, hh)

            pm, phh = prev
            for o2 in range(ot_d):
                d_group(pm, o2, phh, split=(o2 == ot_d - 1))

    nc.compile()
    return nc


def _get_module(t_mega, n_mega, ko_g, ot_g, ot_d, n8g, n8u, n8ua, ga):
    key = (t_mega, n_mega, ko_g, ot_g, ot_d, n8g, n8u, n8ua, ga)
    if key not in _nc_cache:
        _nc_cache[key] = _build_module(
            t_mega, n_mega, ko_g, ot_g, ot_d, n8g, n8u, n8ua, ga
        )
    return _nc_cache[key]


def _prep_x(x, t_mega, n_mega, ko_g, n8x, ko_x):
    """[T, H] f32 -> ([mega, ki, ko_x, t] f16, [mega, ki, n8x, t] f8=8x)."""
    t_total = n_mega * t_mega
    xf = np.ascontiguousarray(x.reshape(t_total, ko_g * 128), dtype=np.float32)
    xr = xf.reshape(n_mega, t_mega, ko_g, 128).transpose(0, 3, 2, 1)
    x_hi = np.ascontiguousarray(xr[:, :, :ko_x, :]).astype(np.float16)
    x8 = (8.0 * np.ascontiguousarray(xr[:, :, ko_g - n8x :, :])).astype(_E4)
    return x_hi, x8


def _prep_w(w_int, ot, ko, n8):
    """[ot*128 (o), ko*128 (k)] int-valued -> fp16 [ot, ki, ko-n8, o] plus
    fp8 [ot, ki, n8, o] (= w/8 on the last n8 k-tiles)."""
    w = w_int.astype(np.float32).reshape(ot, 128, ko, 128).transpose(0, 3, 2, 1)
    w16 = np.ascontiguousarray(w[:, :, : ko - n8, :]).astype(np.float16)
    if n8 == 0:
        return w16, None
    w8 = (np.ascontiguousarray(w[:, :, ko - n8 :, :]) * 0.125).astype(_E4)
    return w16, w8


def _prep_scale(s, ot):
    return np.ascontiguousarray(s.reshape(ot, 128).T, dtype=np.float32)


def _run_spmd(nc, in_maps, trace):
    from concourse.bass_utils import run_bass_kernel_spmd

    return run_bass_kernel_spmd(
        nc, in_maps, core_ids=list(range(len(in_maps))), trace=trace
    )


def kernel(x, gate_wq, gate_scale, up_wq, up_scale, down_wq, down_scale):
    n_mega = T // T_MEGA
    ko_g = H // 128
    ot_g = I_LOC // 128
    ot_d = H // 128
    n8x = max(N8G, N8UA if GA else N8U)
    ko_x = ko_g - min(N8G, N8U)

    nc = _get_module(T_MEGA, n_mega, ko_g, ot_g, ot_d, N8G, N8U, N8UA, GA)

    x_hi, x8 = _prep_x(np.asarray(x), T_MEGA, n_mega, ko_g, n8x, ko_x)
    gate_wq = np.asarray(gate_wq)
    up_wq = np.asarray(up_wq)
    down_wq = np.asarray(down_wq)
    gate_scale = np.asarray(gate_scale, dtype=np.float32)
    up_scale = np.asarray(up_scale, dtype=np.float32)
    down_scale = np.asarray(down_scale, dtype=np.float32)

    in_maps = []
    for c in range(NCORES):
        sl = slice(c * I_LOC, (c + 1) * I_LOC)
        gw16, gw8 = _prep_w(gate_wq[sl], ot_g, ko_g, N8G)
        dw16, _ = _prep_w(down_wq[:, sl], ot_d, ot_g, 0)
        im = {
            "x_hi": x_hi,
            "x8": x8,
            "gate_w": gw16,
            "gate_w8": gw8,
            "down_w": dw16,
            "gate_s": _prep_scale(gate_scale[sl], ot_g),
            "up_s": _prep_scale(up_scale[sl], ot_g),
        }
        up_c = up_wq[sl]
        if GA:
            uwA, uw8A = _prep_w(up_c[: GA * 128], GA, ko_g, N8UA)
            im["up_wA"] = uwA
            im["up_w8A"] = uw8A
        if GA < ot_g:
            uwB, uw8B = _prep_w(up_c[GA * 128 :], ot_g - GA, ko_g, N8U)
            im["up_wB"] = uwB
            im["up_w8B"] = uw8B
        in_maps.append(im)

    trace = bool(int(os.environ.get("TRNMLP_TRACE", "0")))
    res = _run_spmd(nc, in_maps, trace)
    if trace:
        kernel.last_results = res

    acc = res.results[0]["out"].astype(np.float32)
    for r in res.results[1:]:
        acc += r["out"].astype(np.float32)
    acc *= down_scale[:, None]
    return np.ascontiguousarray(acc.T).reshape(B, S, H).astype(np.float32)


kernel.last_results = None
